# revision 8
# baseline (speedup 1.0000x reference)
"""Cross-attention kernel for 8 TRN2 NeuronCores (SPMD, full-I/O contract).

Sharding: 8 cores = 2 batches x 4 head-groups (4 heads each).  Each core
computes its batch's attention for its 4 heads plus the row-sharded slice
of the output projection; the host sums the 4 partial projections per
batch (the "all-reduce") and adds bproj.

Device math (per core, all matmul operands bf16, fp32 PSUM accumulate):
  qT = (Wq_g^T @ x^T  + bq) * HD^-0.5          [256, L]   (feature-major)
  kT =  Wk_g^T @ ctx'^T + bk                   [256, T']
  v  =  ctx'^T-stationary @ Wv_g + bv          [T', 256]
  S^T(tt) = k(tt) @ qT   (2 heads row-packed)  [128, L]
  P^T = exp(S^T + padbias[t])                  (masking is free via bias)
  PV: out^T = [v | 1]^T @ P^T  (M=65: row 64 = softmax sums, for free)
  out^T /= sums;  y_partial = out^T.T @ Wproj_g

ctx is pre-gathered by ctx_mask on the host (~50% of T survives), which
halves every attention-side cost; padded tail positions get bias -1e5 so
exp() underflows to ~0 exactly like the reference's -1e9 masking.
"""

import os
import sys

import numpy as np

B, L, T, D, H = 2, 2048, 2048, 1024, 16
HD = D // H
NCORES = 8
GROUPS = 4          # head-groups (tensor parallel)
DH_CORE = D // GROUPS  # 256 q/k/v dims per core
NEG_BIAS = -100000.0


def _ensure_paths():
    """Make axon site + concourse importable and provide antenv.axon_hooks
    (NTFF profile hook holder) if the image's antenv stub lacks it."""
    defaults = [
        "/root/.axon_site",
        "/root/.axon_site/_ro/trn_rl_repo",
        "/root/.axon_site/_ro/pypackages",
    ]
    for p in reversed(defaults):
        if os.path.isdir(p) and p not in sys.path:
            sys.path.insert(0, p)
    try:
        import antenv.axon_hooks  # noqa: F401
    except ImportError:
        import types

        mod = types.ModuleType("antenv.axon_hooks")
        mod._hook = None

        def set_axon_ntff_profile_hook(hook):
            mod._hook = hook

        def get_axon_ntff_profile_hook():
            return mod._hook

        mod.set_axon_ntff_profile_hook = set_axon_ntff_profile_hook
        mod.get_axon_ntff_profile_hook = get_axon_ntff_profile_hook
        import antenv

        antenv.axon_hooks = mod
        sys.modules["antenv.axon_hooks"] = mod
    import antenv.axon_hooks as ah

    if ah.get_axon_ntff_profile_hook() is None:
        try:
            from trn_agent_boot.trn_boot import _ntff_profile_via_ctypes

            hook = _ntff_profile_via_ctypes("/opt/axon/libaxon_pjrt.so")
            if hook is not None:
                ah.set_axon_ntff_profile_hook(hook)
        except Exception:
            pass


_ensure_paths()

_BUILD_CACHE = {}
LAST_RESULT = None


def build_bass(ntt):
    """Build the SPMD Bass program. ntt = number of 128-row tiles of the
    gathered+padded context length T_pad."""
    from concourse import bacc
    import concourse.bass as bass
    import concourse.mybir as mybir
    import concourse.tile as tile

    T_pad = ntt * 128
    bf = mybir.dt.bfloat16
    f32 = mybir.dt.float32
    EXP = mybir.ActivationFunctionType.Exp

    nc = bacc.Bacc(
        "TRN2",
        target_bir_lowering=False,
        debug=False,
        enable_asserts=False,
        num_devices=NCORES,
    )

    # ---- DRAM I/O (per-core shards, host-prepped) ----
    xT_d = nc.dram_tensor("xT", [D, L], bf, kind="ExternalInput").ap()
    ctxT_d = nc.dram_tensor("ctxT", [D, T_pad], bf, kind="ExternalInput").ap()
    wq_d = nc.dram_tensor("wq", [D, DH_CORE], bf, kind="ExternalInput").ap()
    wk_d = nc.dram_tensor("wk", [D, DH_CORE], bf, kind="ExternalInput").ap()
    wv_d = nc.dram_tensor("wv", [D, DH_CORE], bf, kind="ExternalInput").ap()
    wp_d = nc.dram_tensor("wp", [DH_CORE, D], bf, kind="ExternalInput").ap()
    bq_d = nc.dram_tensor("bq", [2, 128], f32, kind="ExternalInput").ap()
    bk_d = nc.dram_tensor("bk", [2, 128], f32, kind="ExternalInput").ap()
    bv_d = nc.dram_tensor("bv", [1, DH_CORE], bf, kind="ExternalInput").ap()
    mb_d = nc.dram_tensor("mb", [ntt, 128], f32, kind="ExternalInput").ap()
    y_d = nc.dram_tensor("y", [L, D], f32, kind="ExternalOutput").ap()

    with tile.TileContext(nc) as tc:
        import contextlib

        ctx = contextlib.ExitStack()
        with ctx:
            singles = ctx.enter_context(tc.tile_pool(name="singles", bufs=1))
            psA = ctx.enter_context(tc.tile_pool(name="psA", bufs=2, space="PSUM"))
            psB = ctx.enter_context(tc.tile_pool(name="psB", bufs=2, space="PSUM"))
            ppool = ctx.enter_context(tc.tile_pool(name="ppool", bufs=6))
            npool = ctx.enter_context(tc.tile_pool(name="npool", bufs=4))
            ypool = ctx.enter_context(tc.tile_pool(name="ypool", bufs=4))

            # ---- resident inputs ----
            xT = singles.tile([128, 8, L], bf)          # x^T k-tiles
            nc.sync.dma_start(out=xT, in_=xT_d.rearrange("(k p) l -> p k l", p=128))
            ctxT = singles.tile([128, 8, T_pad], bf)    # ctx'^T k-tiles
            nc.sync.dma_start(out=ctxT, in_=ctxT_d.rearrange("(k p) t -> p k t", p=128))
            wq = singles.tile([128, 8, DH_CORE], bf)
            nc.sync.dma_start(out=wq, in_=wq_d.rearrange("(k p) m -> p k m", p=128))
            wk = singles.tile([128, 8, DH_CORE], bf)
            nc.sync.dma_start(out=wk, in_=wk_d.rearrange("(k p) m -> p k m", p=128))
            wv = singles.tile([128, 8, DH_CORE], bf)
            nc.sync.dma_start(out=wv, in_=wv_d.rearrange("(k p) m -> p k m", p=128))
            wp = singles.tile([128, 2, D], bf)          # Wproj rows (2 k-tiles)
            nc.sync.dma_start(out=wp, in_=wp_d.rearrange("(k p) n -> p k n", p=128))
            bq_sb = singles.tile([128, 2], f32)
            nc.sync.dma_start(out=bq_sb, in_=bq_d.rearrange("m p -> p m"))
            bk_sb = singles.tile([128, 2], f32)
            nc.sync.dma_start(out=bk_sb, in_=bk_d.rearrange("m p -> p m"))
            bv_sb = singles.tile([1, DH_CORE], bf)
            nc.sync.dma_start(out=bv_sb, in_=bv_d)
            mb_sb = singles.tile([128, ntt], f32)       # exp bias per t-tile
            nc.sync.dma_start(out=mb_sb, in_=mb_d.rearrange("t p -> p t"))
            ones1 = singles.tile([1, 128], bf)
            nc.vector.memset(ones1, 1.0)

            # ---- residents produced on device ----
            qT = [singles.tile([128, L], bf, name=f"qT{p}", tag=f"qT{p}") for p in range(2)]
            kT = [singles.tile([128, T_pad], bf, name=f"kT{p}", tag=f"kT{p}") for p in range(2)]
            # v with ones column appended per (t-tile, head): [128, ntt, 4, 65]
            v1 = singles.tile([128, ntt, 4, HD + 1], bf)
            nc.vector.memset(v1[:, :, :, HD : HD + 1], 1.0)
            outT = [singles.tile([128, L], bf, name=f"outT{p}", tag=f"outT{p}") for p in range(2)]

            SCALE = float(HD) ** -0.5

            # ---- q^T = Wq^T @ x^T (+bq, *scale) ----
            for m in range(2):
                for lc in range(4):
                    pool = psA if (m * 4 + lc) % 2 == 0 else psB
                    acc = pool.tile([128, 1024], f32, name="qkacc", tag="ps")[:, 0:512]
                    for k in range(8):
                        nc.tensor.matmul(
                            acc,
                            wq[:, k, m * 128 : (m + 1) * 128],
                            xT[:, k, lc * 512 : (lc + 1) * 512],
                            start=(k == 0),
                            stop=(k == 7),
                        )
                    nc.vector.tensor_scalar(
                        out=qT[m][:, lc * 512 : (lc + 1) * 512],
                        in0=acc,
                        scalar1=bq_sb[:, m : m + 1],
                        scalar2=SCALE,
                        op0=mybir.AluOpType.add,
                        op1=mybir.AluOpType.mult,
                    )

            # ---- k^T = Wk^T @ ctx'^T (+bk) ----
            tch = []
            t0 = 0
            while t0 < T_pad:
                tch.append((t0, min(512, T_pad - t0)))
                t0 += 512
            for m in range(2):
                for ci, (tc0, tw) in enumerate(tch):
                    pool = psA if (m * len(tch) + ci) % 2 == 0 else psB
                    acc = pool.tile([128, 1024], f32, name="qkacc", tag="ps")[:, 0:512]
                    for k in range(8):
                        nc.tensor.matmul(
                            acc[:, :tw],
                            wk[:, k, m * 128 : (m + 1) * 128],
                            ctxT[:, k, tc0 : tc0 + tw],
                            start=(k == 0),
                            stop=(k == 7),
                        )
                    nc.vector.tensor_scalar(
                        out=kT[m][:, tc0 : tc0 + tw],
                        in0=acc[:, :tw],
                        scalar1=bk_sb[:, m : m + 1],
                        scalar2=None,
                        op0=mybir.AluOpType.add,
                    )

            # ---- v = ctx' @ Wv (+bv), laid out per (t-tile, head) ----
            for tt in range(ntt):
                pool = psA if tt % 2 == 0 else psB
                acc = pool.tile([128, 1024], f32, name="vacc", tag="ps")[:, 0:DH_CORE]
                for k in range(8):
                    nc.tensor.matmul(
                        acc,
                        ctxT[:, k, tt * 128 : (tt + 1) * 128],
                        wv[:, k, :],
                        start=(k == 0),
                        stop=False,
                    )
                # += 1 @ bv  (broadcast bias over the 128 t-rows)
                nc.tensor.matmul(acc, ones1, bv_sb, start=False, stop=True)
                for h in range(4):
                    nc.vector.tensor_copy(
                        v1[:, tt, h, 0:HD], acc[:, h * HD : (h + 1) * HD]
                    )

            # ---- attention: per (head-pair, l-half) ----
            for p in range(2):
                for hf in range(2):
                    lo = hf * 1024
                    Sr = [psA.tile([128, 1024], f32, name=f"S{h}", tag="ps") for h in range(2)]
                    pv = [
                        psB.tile([128, 1024], f32, name=f"pv{h}", tag="ps")
                        for h in range(2)
                    ]
                    for tt in range(ntt):
                        # scores: 2 heads row-packed (rows 0-63 / 64-127)
                        for lc in range(2):
                            for h in range(2):
                                nc.tensor.matmul(
                                    Sr[h][:, lc * 512 : (lc + 1) * 512],
                                    kT[p][h * 64 : (h + 1) * 64, tt * 128 : (tt + 1) * 128],
                                    qT[p][h * 64 : (h + 1) * 64, lo + lc * 512 : lo + (lc + 1) * 512],
                                    start=True,
                                    stop=True,
                                )
                        # exp with padding bias (per-partition = per-t)
                        pt = [ppool.tile([128, 1024], bf, name=f"P{h2}", tag="P") for h2 in range(2)]
                        for h in range(2):
                            nc.scalar.activation(
                                pt[h], Sr[h], EXP, bias=mb_sb[:, tt : tt + 1]
                            )
                        # PV accumulate; ones column makes row 64 the softmax sum
                        for h in range(2):
                            for lc in range(2):
                                nc.tensor.matmul(
                                    pv[h][0 : HD + 1, lc * 512 : (lc + 1) * 512],
                                    v1[:, tt, p * 2 + h, :],
                                    pt[h][:, lc * 512 : (lc + 1) * 512],
                                    start=(tt == 0),
                                    stop=(tt == ntt - 1),
                                )
                    # normalize: out^T[d, l] * (1 / sums[l]) -> bf16 resident
                    for h in range(2):
                        rec1 = npool.tile([1, 1024], f32, name="rec1", tag="rec1")
                        nc.vector.reciprocal(rec1, pv[h][HD : HD + 1, :])
                        rec = npool.tile([64, 1024], f32, name="rec", tag="rec")
                        nc.gpsimd.partition_broadcast(rec, rec1)
                        nc.vector.tensor_mul(
                            outT[p][h * 64 : (h + 1) * 64, lo : lo + 1024],
                            pv[h][0:HD, :],
                            rec,
                        )

            # ---- y_partial = out^T.T @ Wproj_g ----
            for lt in range(16):
                yt = ypool.tile([128, D], f32, tag="yt")
                for nk in range(2):
                    pool = psA if nk % 2 == 0 else psB
                    acc = pool.tile([128, 1024], f32, name="yacc", tag="ps")[:, 0:512]
                    for p in range(2):
                        nc.tensor.matmul(
                            acc,
                            outT[p][:, lt * 128 : (lt + 1) * 128],
                            wp[:, p, nk * 512 : (nk + 1) * 512],
                            start=(p == 0),
                            stop=(p == 1),
                        )
                    nc.vector.tensor_copy(yt[:, nk * 512 : (nk + 1) * 512], acc)
                nc.sync.dma_start(
                    out=y_d[lt * 128 : (lt + 1) * 128, :], in_=yt
                )

    nc.compile()
    return nc


def kernel(x, ctx, ctx_mask, Wq, bq, Wkv, bkv, Wproj, bproj):
    import ml_dtypes

    x = np.asarray(x, np.float32)
    ctx = np.asarray(ctx, np.float32)
    ctx_mask = np.asarray(ctx_mask)
    Wq = np.asarray(Wq, np.float32)
    bq = np.asarray(bq, np.float32)
    Wkv = np.asarray(Wkv, np.float32)
    bkv = np.asarray(bkv, np.float32)
    Wproj = np.asarray(Wproj, np.float32)
    bproj = np.asarray(bproj, np.float32)
    assert x.shape == (B, L, D) and ctx.shape == (B, T, D)

    bff = ml_dtypes.bfloat16

    # gather context by mask per batch; common padded length for SPMD
    idxs = [np.flatnonzero(ctx_mask[b]) for b in range(B)]
    tmax = max(1, max(len(i) for i in idxs))
    ntt = (tmax + 127) // 128
    T_pad = ntt * 128

    key = ntt
    if key not in _BUILD_CACHE:
        _BUILD_CACHE[key] = build_bass(ntt)
    nc = _BUILD_CACHE[key]

    in_maps = []
    for core in range(NCORES):
        b, g = core // GROUPS, core % GROUPS
        idx = idxs[b]
        tp = len(idx)
        ctxg = np.zeros((T_pad, D), np.float32)
        ctxg[:tp] = ctx[b][idx]
        mb = np.full(T_pad, NEG_BIAS, np.float32)
        mb[:tp] = 0.0
        s = slice(g * DH_CORE, (g + 1) * DH_CORE)
        in_maps.append(
            {
                "xT": np.ascontiguousarray(x[b].T).astype(bff),
                "ctxT": np.ascontiguousarray(ctxg.T).astype(bff),
                "wq": np.ascontiguousarray(Wq[:, s]).astype(bff),
                "wk": np.ascontiguousarray(Wkv[:, s]).astype(bff),
                "wv": np.ascontiguousarray(Wkv[:, D + g * DH_CORE : D + (g + 1) * DH_CORE]).astype(bff),
                "wp": np.ascontiguousarray(Wproj[s, :]).astype(bff),
                "bq": np.ascontiguousarray(bq[s].reshape(2, 128)),
                "bk": np.ascontiguousarray(bkv[s].reshape(2, 128)),
                "bv": np.ascontiguousarray(
                    bkv[D + g * DH_CORE : D + (g + 1) * DH_CORE].reshape(1, -1)
                ).astype(bff),
                "mb": np.ascontiguousarray(mb.reshape(ntt, 128)),
            }
        )

    from concourse.bass_utils import run_bass_kernel_spmd

    trace = bool(os.environ.get("KBENCH_TRACE"))
    res = run_bass_kernel_spmd(nc, in_maps, core_ids=list(range(NCORES)), trace=trace)
    global LAST_RESULT
    LAST_RESULT = res

    y = np.zeros((B, L, D), np.float32)
    for core in range(NCORES):
        y[core // GROUPS] += res.results[core]["y"]
    y += bproj[None, None, :]
    return y


# revision 10
# speedup vs baseline: 1.3577x; 1.3577x over previous
"""Cross-attention kernel for 8 TRN2 NeuronCores (SPMD, full-I/O contract).

Sharding: 8 cores = 2 batches x 4 head-groups (4 heads each).  Each core
computes its batch's attention for its 4 heads plus the row-sharded slice
of the output projection; the host sums the 4 partial projections per
batch (the "all-reduce") and adds bproj.

Device math (per core, all matmul operands bf16, fp32 PSUM accumulate):
  qT = (Wq_g^T @ x^T  + bq) * HD^-0.5          [256, L]   (feature-major)
  kT =  Wk_g^T @ ctx'^T + bk                   [256, T']
  v  =  ctx'^T-stationary @ Wv_g + bv          [T', 256]
  S^T(tt) = k(tt) @ qT   (2 heads row-packed)  [128, L]
  P^T = exp(S^T + padbias[t])                  (masking is free via bias)
  PV: out^T = [v | 1]^T @ P^T  (M=65: row 64 = softmax sums, for free)
  out^T /= sums;  y_partial = out^T.T @ Wproj_g

ctx is pre-gathered by ctx_mask on the host (~50% of T survives), which
halves every attention-side cost; padded tail positions get bias -1e5 so
exp() underflows to ~0 exactly like the reference's -1e9 masking.
"""

import os
import sys

import numpy as np

B, L, T, D, H = 2, 2048, 2048, 1024, 16
HD = D // H
NCORES = 8
GROUPS = 4          # head-groups (tensor parallel)
DH_CORE = D // GROUPS  # 256 q/k/v dims per core
NEG_BIAS = -100000.0


def _ensure_paths():
    """Make axon site + concourse importable and provide antenv.axon_hooks
    (NTFF profile hook holder) if the image's antenv stub lacks it."""
    defaults = [
        "/root/.axon_site",
        "/root/.axon_site/_ro/trn_rl_repo",
        "/root/.axon_site/_ro/pypackages",
    ]
    for p in reversed(defaults):
        if os.path.isdir(p) and p not in sys.path:
            sys.path.insert(0, p)
    try:
        import antenv.axon_hooks  # noqa: F401
    except ImportError:
        import types

        mod = types.ModuleType("antenv.axon_hooks")
        mod._hook = None

        def set_axon_ntff_profile_hook(hook):
            mod._hook = hook

        def get_axon_ntff_profile_hook():
            return mod._hook

        mod.set_axon_ntff_profile_hook = set_axon_ntff_profile_hook
        mod.get_axon_ntff_profile_hook = get_axon_ntff_profile_hook
        import antenv

        antenv.axon_hooks = mod
        sys.modules["antenv.axon_hooks"] = mod
    import antenv.axon_hooks as ah

    if ah.get_axon_ntff_profile_hook() is None:
        try:
            from trn_agent_boot.trn_boot import _ntff_profile_via_ctypes

            hook = _ntff_profile_via_ctypes("/opt/axon/libaxon_pjrt.so")
            if hook is not None:
                ah.set_axon_ntff_profile_hook(hook)
        except Exception:
            pass


_ensure_paths()

_BUILD_CACHE = {}
LAST_RESULT = None


def build_bass(ntt):
    """Build the SPMD Bass program. ntt = number of 128-row tiles of the
    gathered+padded context length T_pad."""
    from concourse import bacc
    import concourse.bass as bass
    import concourse.mybir as mybir
    import concourse.tile as tile

    T_pad = ntt * 128
    bf = mybir.dt.bfloat16
    f32 = mybir.dt.float32
    EXP = mybir.ActivationFunctionType.Exp

    nc = bacc.Bacc(
        "TRN2",
        target_bir_lowering=False,
        debug=False,
        enable_asserts=False,
        num_devices=NCORES,
    )

    # ---- DRAM I/O (per-core shards, host-prepped) ----
    xT_d = nc.dram_tensor("xT", [D, L], bf, kind="ExternalInput").ap()
    ctxT_d = nc.dram_tensor("ctxT", [D, T_pad], bf, kind="ExternalInput").ap()
    wq_d = nc.dram_tensor("wq", [D, DH_CORE], bf, kind="ExternalInput").ap()
    wk_d = nc.dram_tensor("wk", [D, DH_CORE], bf, kind="ExternalInput").ap()
    wv_d = nc.dram_tensor("wv", [D, DH_CORE], bf, kind="ExternalInput").ap()
    wp_d = nc.dram_tensor("wp", [DH_CORE, D], bf, kind="ExternalInput").ap()
    bq_d = nc.dram_tensor("bq", [2, 128], f32, kind="ExternalInput").ap()
    bk_d = nc.dram_tensor("bk", [2, 128], f32, kind="ExternalInput").ap()
    bv_d = nc.dram_tensor("bv", [1, DH_CORE], bf, kind="ExternalInput").ap()
    mb_d = nc.dram_tensor("mb", [ntt, 128], f32, kind="ExternalInput").ap()
    y_d = nc.dram_tensor("y", [L, D], f32, kind="ExternalOutput").ap()

    with tile.TileContext(nc) as tc:
        import contextlib

        ctx = contextlib.ExitStack()
        with ctx:
            singles = ctx.enter_context(tc.tile_pool(name="singles", bufs=1))
            psA = ctx.enter_context(tc.tile_pool(name="psA", bufs=2, space="PSUM"))
            psB = ctx.enter_context(tc.tile_pool(name="psB", bufs=2, space="PSUM"))
            ppool = ctx.enter_context(tc.tile_pool(name="ppool", bufs=6))
            npool = ctx.enter_context(tc.tile_pool(name="npool", bufs=4))
            ypool = ctx.enter_context(tc.tile_pool(name="ypool", bufs=4))

            # ---- resident inputs ----
            xT = singles.tile([128, 8, L], bf)          # x^T k-tiles
            xTr = xT_d.rearrange("(k p) l -> k p l", p=128)
            for k in range(8):
                nc.sync.dma_start(out=xT[:, k, :], in_=xTr[k])
            ctxT = singles.tile([128, 8, T_pad], bf)    # ctx'^T k-tiles
            ctxTr = ctxT_d.rearrange("(k p) t -> k p t", p=128)
            for k in range(8):
                nc.sync.dma_start(out=ctxT[:, k, :], in_=ctxTr[k])
            wq = singles.tile([128, 8, DH_CORE], bf)
            nc.sync.dma_start(out=wq, in_=wq_d.rearrange("(k p) m -> p k m", p=128))
            wk = singles.tile([128, 8, DH_CORE], bf)
            nc.sync.dma_start(out=wk, in_=wk_d.rearrange("(k p) m -> p k m", p=128))
            wv = singles.tile([128, 8, DH_CORE], bf)
            nc.sync.dma_start(out=wv, in_=wv_d.rearrange("(k p) m -> p k m", p=128))
            wp = singles.tile([128, 2, D], bf)          # Wproj rows (2 k-tiles)
            nc.sync.dma_start(out=wp, in_=wp_d.rearrange("(k p) n -> p k n", p=128))
            bq_sb = singles.tile([128, 2], f32)
            nc.sync.dma_start(out=bq_sb, in_=bq_d.rearrange("m p -> p m"))
            bk_sb = singles.tile([128, 2], f32)
            nc.sync.dma_start(out=bk_sb, in_=bk_d.rearrange("m p -> p m"))
            bv_sb = singles.tile([1, DH_CORE], bf)
            nc.sync.dma_start(out=bv_sb, in_=bv_d)
            mb_sb = singles.tile([128, ntt], f32)       # exp bias per t-tile
            nc.sync.dma_start(out=mb_sb, in_=mb_d.rearrange("t p -> p t"))
            ones1 = singles.tile([1, 128], bf)
            nc.vector.memset(ones1, 1.0)

            # ---- residents produced on device ----
            qT = [singles.tile([128, L], bf, name=f"qT{p}", tag=f"qT{p}") for p in range(2)]
            kT = [singles.tile([128, T_pad], bf, name=f"kT{p}", tag=f"kT{p}") for p in range(2)]
            # v with ones column appended per (t-tile, head): [128, ntt, 4, 65]
            v1 = singles.tile([128, ntt, 4, HD + 1], bf)
            nc.vector.memset(v1[:, :, :, HD : HD + 1], 1.0)
            outT = [singles.tile([128, L], bf, name=f"outT{p}", tag=f"outT{p}") for p in range(2)]

            SCALE = float(HD) ** -0.5

            # ---- q^T = Wq^T @ x^T (+bq, *scale) ----
            for m in range(2):
                for lc in range(4):
                    pool = psA if (m * 4 + lc) % 2 == 0 else psB
                    acc = pool.tile([128, 1024], f32, name="qkacc", tag="ps")[:, 0:512]
                    for k in range(8):
                        nc.tensor.matmul(
                            acc,
                            wq[:, k, m * 128 : (m + 1) * 128],
                            xT[:, k, lc * 512 : (lc + 1) * 512],
                            start=(k == 0),
                            stop=(k == 7),
                        )
                    nc.vector.tensor_scalar(
                        out=qT[m][:, lc * 512 : (lc + 1) * 512],
                        in0=acc,
                        scalar1=bq_sb[:, m : m + 1],
                        scalar2=SCALE,
                        op0=mybir.AluOpType.add,
                        op1=mybir.AluOpType.mult,
                    )

            # ---- k^T = Wk^T @ ctx'^T (+bk) ----
            tch = []
            t0 = 0
            while t0 < T_pad:
                tch.append((t0, min(512, T_pad - t0)))
                t0 += 512
            for m in range(2):
                for ci, (tc0, tw) in enumerate(tch):
                    pool = psA if (m * len(tch) + ci) % 2 == 0 else psB
                    acc = pool.tile([128, 1024], f32, name="qkacc", tag="ps")[:, 0:512]
                    for k in range(8):
                        nc.tensor.matmul(
                            acc[:, :tw],
                            wk[:, k, m * 128 : (m + 1) * 128],
                            ctxT[:, k, tc0 : tc0 + tw],
                            start=(k == 0),
                            stop=(k == 7),
                        )
                    nc.vector.tensor_scalar(
                        out=kT[m][:, tc0 : tc0 + tw],
                        in0=acc[:, :tw],
                        scalar1=bk_sb[:, m : m + 1],
                        scalar2=None,
                        op0=mybir.AluOpType.add,
                    )

            # ---- v = ctx' @ Wv (+bv), laid out per (t-tile, head) ----
            for tt in range(ntt):
                pool = psA if tt % 2 == 0 else psB
                acc = pool.tile([128, 1024], f32, name="vacc", tag="ps")[:, 0:DH_CORE]
                for k in range(8):
                    nc.tensor.matmul(
                        acc,
                        ctxT[:, k, tt * 128 : (tt + 1) * 128],
                        wv[:, k, :],
                        start=(k == 0),
                        stop=False,
                    )
                # += 1 @ bv  (broadcast bias over the 128 t-rows)
                nc.tensor.matmul(acc, ones1, bv_sb, start=False, stop=True)
                for h in range(4):
                    nc.vector.tensor_copy(
                        v1[:, tt, h, 0:HD], acc[:, h * HD : (h + 1) * HD]
                    )

            # ---- attention: per (head-pair, l-half) ----
            for p in range(2):
                for hf in range(2):
                    lo = hf * 1024
                    Sr = [psA.tile([128, 1024], f32, name=f"S{h}", tag="ps") for h in range(2)]
                    pv = [
                        psB.tile([128, 1024], f32, name=f"pv{h}", tag="ps")
                        for h in range(2)
                    ]
                    for tt in range(ntt):
                        # scores: 2 heads row-packed (rows 0-63 / 64-127)
                        for h in range(2):
                            for lc in range(2):
                                nc.tensor.matmul(
                                    Sr[h][:, lc * 512 : (lc + 1) * 512],
                                    kT[p][h * 64 : (h + 1) * 64, tt * 128 : (tt + 1) * 128],
                                    qT[p][h * 64 : (h + 1) * 64, lo + lc * 512 : lo + (lc + 1) * 512],
                                    start=True,
                                    stop=True,
                                )
                        # exp with padding bias (per-partition = per-t)
                        pt = [ppool.tile([128, 1024], bf, name=f"P{h2}", tag="P") for h2 in range(2)]
                        for h in range(2):
                            nc.scalar.activation(
                                pt[h], Sr[h], EXP, bias=mb_sb[:, tt : tt + 1]
                            )
                        # PV accumulate; ones column makes row 64 the softmax sum
                        for h in range(2):
                            for lc in range(2):
                                nc.tensor.matmul(
                                    pv[h][0 : HD + 1, lc * 512 : (lc + 1) * 512],
                                    v1[:, tt, p * 2 + h, :],
                                    pt[h][:, lc * 512 : (lc + 1) * 512],
                                    start=(tt == 0),
                                    stop=(tt == ntt - 1),
                                )
                    # normalize: out^T[d, l] * (1 / sums[l]) -> bf16 resident
                    for h in range(2):
                        srow = npool.tile([1, 1024], f32, name="srow", tag="srow")
                        rec1 = npool.tile([1, 1024], f32, name="rec1", tag="rec1")
                        nc.vector.tensor_copy(srow, pv[h][HD : HD + 1, :])
                        nc.vector.reciprocal_approx_fast(rec1, srow)
                        rec = npool.tile([64, 1024], f32, name="rec", tag="rec")
                        nc.gpsimd.partition_broadcast(rec, rec1)
                        nc.vector.tensor_mul(
                            outT[p][h * 64 : (h + 1) * 64, lo : lo + 1024],
                            pv[h][0:HD, :],
                            rec,
                        )

            # ---- y_partial = out^T.T @ Wproj_g ----
            for lt in range(16):
                yt = ypool.tile([128, D], f32, tag="yt")
                for nk in range(2):
                    pool = psA if nk % 2 == 0 else psB
                    acc = pool.tile([128, 1024], f32, name="yacc", tag="ps")[:, 0:512]
                    for p in range(2):
                        nc.tensor.matmul(
                            acc,
                            outT[p][:, lt * 128 : (lt + 1) * 128],
                            wp[:, p, nk * 512 : (nk + 1) * 512],
                            start=(p == 0),
                            stop=(p == 1),
                        )
                    nc.vector.tensor_copy(yt[:, nk * 512 : (nk + 1) * 512], acc)
                nc.sync.dma_start(
                    out=y_d[lt * 128 : (lt + 1) * 128, :], in_=yt
                )

    nc.compile()
    return nc


def kernel(x, ctx, ctx_mask, Wq, bq, Wkv, bkv, Wproj, bproj):
    import ml_dtypes

    x = np.asarray(x, np.float32)
    ctx = np.asarray(ctx, np.float32)
    ctx_mask = np.asarray(ctx_mask)
    Wq = np.asarray(Wq, np.float32)
    bq = np.asarray(bq, np.float32)
    Wkv = np.asarray(Wkv, np.float32)
    bkv = np.asarray(bkv, np.float32)
    Wproj = np.asarray(Wproj, np.float32)
    bproj = np.asarray(bproj, np.float32)
    assert x.shape == (B, L, D) and ctx.shape == (B, T, D)

    bff = ml_dtypes.bfloat16

    # gather context by mask per batch; common padded length for SPMD
    idxs = [np.flatnonzero(ctx_mask[b]) for b in range(B)]
    tmax = max(1, max(len(i) for i in idxs))
    ntt = (tmax + 127) // 128
    T_pad = ntt * 128

    key = ntt
    if key not in _BUILD_CACHE:
        _BUILD_CACHE[key] = build_bass(ntt)
    nc = _BUILD_CACHE[key]

    in_maps = []
    for core in range(NCORES):
        b, g = core // GROUPS, core % GROUPS
        idx = idxs[b]
        tp = len(idx)
        ctxg = np.zeros((T_pad, D), np.float32)
        ctxg[:tp] = ctx[b][idx]
        mb = np.full(T_pad, NEG_BIAS, np.float32)
        mb[:tp] = 0.0
        s = slice(g * DH_CORE, (g + 1) * DH_CORE)
        in_maps.append(
            {
                "xT": np.ascontiguousarray(x[b].T).astype(bff),
                "ctxT": np.ascontiguousarray(ctxg.T).astype(bff),
                "wq": np.ascontiguousarray(Wq[:, s]).astype(bff),
                "wk": np.ascontiguousarray(Wkv[:, s]).astype(bff),
                "wv": np.ascontiguousarray(Wkv[:, D + g * DH_CORE : D + (g + 1) * DH_CORE]).astype(bff),
                "wp": np.ascontiguousarray(Wproj[s, :]).astype(bff),
                "bq": np.ascontiguousarray(bq[s].reshape(2, 128)),
                "bk": np.ascontiguousarray(bkv[s].reshape(2, 128)),
                "bv": np.ascontiguousarray(
                    bkv[D + g * DH_CORE : D + (g + 1) * DH_CORE].reshape(1, -1)
                ).astype(bff),
                "mb": np.ascontiguousarray(mb.reshape(ntt, 128)),
            }
        )

    from concourse.bass_utils import run_bass_kernel_spmd

    trace = bool(os.environ.get("KBENCH_TRACE"))
    res = run_bass_kernel_spmd(nc, in_maps, core_ids=list(range(NCORES)), trace=trace)
    global LAST_RESULT
    LAST_RESULT = res

    y = np.zeros((B, L, D), np.float32)
    for core in range(NCORES):
        y[core // GROUPS] += res.results[core]["y"]
    y += bproj[None, None, :]
    return y
